# revision 17
# baseline (speedup 1.0000x reference)
"""Causal multi-head self-attention (B=1, S=4096, D=1024, H=16) on 8 NeuronCores.

Sharding: tensor-parallel over heads — each core owns 2 heads (Wq/Wk/Wv column
slices, Wo row slice), computes a partial output projection, and the host sums
the 8 partials.

Device-side design (per core):
  - Host pre-transposes x -> xT [D, S] and pre-permutes/stacks weights so no
    on-device layout shuffles are needed for q/k.
  - qT/kT computed in [channel, seq] layout (rows = [h0_x1|h1_x1|h0_x2|h1_x2],
    channels de-interleaved even/odd so RoPE is two contiguous 64-row halves).
  - v computed in [ch, seq] then PE-transposed to [seq, ch] blocks (PV matmul
    needs V as the stationary operand in natural layout).
  - Attention in scores^T layout: scores^T[sk,sq] = K_blk @ Q_blk^T, softmax
    without max-subtraction (scores are O(±8) for this distribution so exp is
    safe in fp32), row-sums obtained by appending a ones-column to V, causal
    masking via one static 128x128 triangle tile + gpsimd memsets on fully
    masked column ranges.
  - All matmuls run as float32r (full PE rate, fp32 storage).
"""

import os
import sys

import numpy as np

for _p in ("/opt/trn_rl_repo", "/root/.axon_site/_ro/trn_rl_repo"):
    if os.path.isdir(_p) and _p not in sys.path:
        sys.path.insert(0, _p)

import concourse.bass as bass
import concourse.mybir as mybir
import concourse.tile as tile
from concourse import bacc
from concourse.bass_utils import run_bass_kernel_spmd
from concourse.masks import make_identity


def _install_ntff_shim():
    """The agent image's antenv lacks axon_hooks; provide it so
    run_bass_kernel_spmd(trace=True) can capture NTFF profiles."""
    try:
        from antenv import axon_hooks  # noqa: F401
        return
    except ImportError:
        pass
    try:
        import types
        import antenv
        from trn_agent_boot.trn_boot import _ntff_profile_via_ctypes
        so = "/opt/axon/libaxon_pjrt.so"
        if not os.path.exists(so):
            return
        hook = _ntff_profile_via_ctypes(so)
        m = types.ModuleType("antenv.axon_hooks")
        m._hook = hook
        m.set_axon_ntff_profile_hook = lambda h: setattr(m, "_hook", h)
        m.get_axon_ntff_profile_hook = lambda: m._hook
        sys.modules["antenv.axon_hooks"] = m
        antenv.axon_hooks = m
    except Exception:
        pass


_install_ntff_shim()

F32 = mybir.dt.float32
F32R = mybir.dt.float32r

S = 4096
D = 1024
H = 16
DK = 64
N_CORES = 8
SQ = 512          # q-block width (PSUM bank limit for fp32 N)
SK = 128          # k-block width (partition dim of scores^T)
GK = 3            # k-blocks fused per exp group (3 PSUM banks)
NB512 = S // SQ   # 8
NB128 = S // SK   # 32


def _emit(tc, xT, wqkT, wvT, woT, cosT, sinT, tri, y):
    nc = tc.nc
    ctx_pools = []
    PHASES = int(os.environ.get("KERN_PHASES", "5"))

    # ---------------- persistent SBUF ----------------
    const = tc.tile_pool(name="const", bufs=1)
    big = tc.tile_pool(name="big", bufs=1)
    cp = const.__enter__()
    bp = big.__enter__()
    ctx_pools += [const, big]

    wqk_sb = cp.tile([128, 8, 256], F32R, tag="wqk")      # [part, kchunk, 256]
    wv_sb = cp.tile([128, 8, 128], F32R, tag="wv")
    wo_sb = cp.tile([128, 1024], F32R, tag="wo")
    cos_sb = cp.tile([128, S], F32, tag="cos")
    sin_sb = cp.tile([128, S], F32, tag="sin")
    tri_sb = cp.tile([128, 512], F32, tag="tri")
    ident = cp.tile([128, 128], F32, tag="ident")
    ones_sb = cp.tile([65, 64], F32R, tag="ones")

    nc.sync.dma_start(wqk_sb[:], wqkT.ap().rearrange("(c p) n -> p c n", p=128))
    nc.sync.dma_start(wv_sb[:], wvT.ap().rearrange("(c p) n -> p c n", p=128))
    nc.sync.dma_start(wo_sb[:], woT.ap())
    nc.sync.dma_start(cos_sb[:], cosT.ap())
    nc.sync.dma_start(sin_sb[:], sinT.ap())
    nc.sync.dma_start(tri_sb[:], tri.ap())
    make_identity(nc, ident[:])
    ones_f32 = cp.tile([65, 64], F32, tag="ones_f32")
    nc.vector.memset(ones_f32[:], 1.0)
    nc.vector.tensor_copy(ones_sb[64:65, :], ones_f32[64:65, :])

    qT = bp.tile([128, S], F32R, tag="qT")
    kT = bp.tile([128, S], F32R, tag="kT")
    vt_sb = bp.tile([128, S], F32, tag="vt_out")         # vT, later reused for outT
    v_h = [bp.tile([128, NB128 * 65], F32R, tag=f"v{h}", name=f"v{h}")
           for h in range(2)]

    # ---------------- phase B: projections ----------------
    with tc.tile_pool(name="xp", bufs=12) as xp, \
         tc.tile_pool(name="prps", bufs=6, space="PSUM") as prps:
        for sb in range(NB512):
            xts = []
            for kc in range(8):
                xt = xp.tile([128, SQ], F32R, tag="x")
                nc.sync.dma_start(
                    xt[:], xT.ap()[128 * kc:128 * (kc + 1), SQ * sb:SQ * (sb + 1)])
                xts.append(xt)
            q_ps = prps.tile([128, SQ], F32, tag="pr")
            k_ps = prps.tile([128, SQ], F32, tag="pr")
            v_ps = prps.tile([128, SQ], F32, tag="pr")
            for kc in range(8):
                st, sp = (kc == 0), (kc == 7)
                nc.tensor.matmul(q_ps[:], wqk_sb[:, kc, 0:128], xts[kc][:],
                                 start=st, stop=sp)
                nc.tensor.matmul(k_ps[:], wqk_sb[:, kc, 128:256], xts[kc][:],
                                 start=st, stop=sp)
                nc.tensor.matmul(v_ps[:], wv_sb[:, kc, :], xts[kc][:],
                                 start=st, stop=sp)
            sl = slice(SQ * sb, SQ * (sb + 1))
            nc.vector.tensor_copy(qT[:, sl], q_ps[:])
            nc.vector.tensor_copy(kT[:, sl], k_ps[:])
            nc.vector.tensor_copy(vt_sb[:, sl], v_ps[:])

    if PHASES < 2:
        for p in reversed(ctx_pools):
            p.__exit__(None, None, None)
        return
    # ---------------- RoPE (in place on qT, kT) ----------------
    # Row layout: [h0_x1(32) | h0_x2(32) | h1_x1(32) | h1_x2(32)] so each
    # head's d_k=64 rows are contiguous for the attention matmuls.
    # sw = 32-row swap within each head block, so every op is partition-local:
    #   a = t  * cos2   -> per block: [X1c; X2c]
    #   b = sw * sin2   -> per block: [X2s; X1s]
    #   x1 rows <- a - b;  x2 rows <- b + a
    mul = mybir.AluOpType.mult
    with tc.tile_pool(name="rope", bufs=2) as rp:
        HW = 2048  # free-dim chunk
        for t in (qT, kT):
            sw = rp.tile([128, S], F32R, tag="swp", bufs=1)
            for blk in range(4):
                dst = slice(32 * blk, 32 * blk + 32)
                srcs = slice(32 * (blk ^ 1), 32 * (blk ^ 1) + 32)
                nc.sync.dma_start(sw[dst, :], t[srcs, :])
            for c0 in range(0, S, HW):
                sl = slice(c0, c0 + HW)
                ta = rp.tile([128, HW], F32, tag="ta")
                tb = rp.tile([128, HW], F32, tag="tb")
                nc.vector.tensor_tensor(ta[:], t[:, sl], cos_sb[:, sl], op=mul)
                nc.vector.tensor_tensor(tb[:], sw[:, sl], sin_sb[:, sl], op=mul)
                for hb in range(2):
                    x1 = slice(64 * hb, 64 * hb + 32)
                    x2 = slice(64 * hb + 32, 64 * hb + 64)
                    nc.vector.tensor_tensor(t[x1, sl], ta[x1, :], tb[x1, :],
                                            op=mybir.AluOpType.subtract)
                    nc.vector.tensor_tensor(t[x2, sl], tb[x2, :], ta[x2, :],
                                            op=mybir.AluOpType.add)

    if PHASES < 3:
        for p in reversed(ctx_pools):
            p.__exit__(None, None, None)
        return
    # ---------------- phase C: v^T -> v blocks (PE transpose) ----------------
    # ones column for row-sums: v_h[:, 65j+64] = 1.0
    onec = cp.tile([128, 1], F32, tag="onec")
    nc.vector.memset(onec[:], 1.0)
    for h in range(2):
        ones_col = v_h[h][:].rearrange("p (b c) -> p b c", c=65)[:, :, 64]
        nc.vector.tensor_copy(ones_col, onec[:].broadcast_to([128, 32]))
    with tc.tile_pool(name="vtp", bufs=4, space="PSUM") as vtp:
        for j in range(NB128):
            tp = vtp.tile([128, 128], F32, tag="vt")
            nc.tensor.transpose(tp[:], vt_sb[:, 128 * j:128 * (j + 1)], ident[:])
            for h in range(2):
                nc.vector.tensor_copy(v_h[h][:, 65 * j:65 * j + 64],
                                      tp[:, 64 * h:64 * h + 64])

    if PHASES < 4:
        for p in reversed(ctx_pools):
            p.__exit__(None, None, None)
        return
    # ---------------- phase D: attention ----------------
    DMODE = int(os.environ.get("KERN_DMODE", "4"))
    outT = bp.tile([128, S], F32R, tag="vt_out")  # reuses vT slot
    exp = mybir.ActivationFunctionType.Exp
    with tc.tile_pool(name="scps", bufs=2, space="PSUM") as scps, \
         tc.tile_pool(name="smps", bufs=2, space="PSUM") as smps, \
         tc.tile_pool(name="ptp", bufs=3) as ptp, \
         tc.tile_pool(name="recp", bufs=2) as recp:
        for b in range(NB512):
            nk = 4 * b + 4
            qsl = slice(SQ * b, SQ * (b + 1))
            for h in range(2):
                out_ps = smps.tile([65, SQ], F32, tag="sm")
                rh = slice(64 * h, 64 * h + 64)          # head h d_k rows
                for g0 in range(0, nk, GK):
                    gw = min(GK, nk - g0)
                    sc = scps.tile([128, GK * SQ], F32, tag="sc")
                    for j2 in range(gw):
                        k = g0 + j2
                        ksl = slice(SK * k, SK * (k + 1))
                        osl = slice(SQ * j2, SQ * (j2 + 1))
                        nc.tensor.matmul(sc[:, osl], kT[rh, ksl], qT[rh, qsl],
                                         start=True, stop=True)
                    pt = ptp.tile([128, GK * SQ], F32R, tag="pt")
                    nc.scalar.activation(pt[:, 0:SQ * gw], sc[:, 0:SQ * gw], exp,
                                         scale=0.125)
                    if DMODE >= 2:
                        for j2 in range(gw):
                            k = g0 + j2
                            if k >= 4 * b:           # diagonal-straddling block
                                j = k - 4 * b
                                c0 = SQ * j2
                                w = 128 * j + 128
                                dsl = slice(c0, c0 + w)
                                nc.vector.tensor_tensor(pt[:, dsl], pt[:, dsl],
                                                        tri_sb[:, 512 - w:512],
                                                        op=mul)
                    if DMODE >= 3:
                        for j2 in range(gw):
                            k = g0 + j2
                            nc.tensor.matmul(out_ps[:],
                                             v_h[h][:, 65 * k:65 * k + 65],
                                             pt[:, SQ * j2:SQ * (j2 + 1)],
                                             start=(k == 0), stop=(k == nk - 1))
                if DMODE < 4:
                    continue
                rec = recp.tile([65, SQ], F32R, tag="rec")
                with nc.allow_low_precision(reason="fp32r reciprocal for bcast"):
                    nc.vector.reciprocal(rec[64:65, :], out_ps[64:65, :])
                bc = smps.tile([64, SQ], F32, tag="sm")
                nc.tensor.matmul(bc[:], ones_sb[64:65, :], rec[64:65, :],
                                 start=True, stop=True)
                if h == 0:
                    dst = outT[0:64, qsl]
                else:
                    tmp64 = recp.tile([64, SQ], F32R, tag="tmp64")
                    dst = tmp64[:]
                nc.vector.tensor_copy(dst, out_ps[0:64, :])
                nc.vector.tensor_tensor(dst, dst, bc[:], op=mul)
                if h == 1:
                    nc.sync.dma_start(outT[64:128, qsl], tmp64[:])

    if PHASES < 5:
        for p in reversed(ctx_pools):
            p.__exit__(None, None, None)
        return
    # ---------------- phase E: output projection ----------------
    with tc.tile_pool(name="yps", bufs=2, space="PSUM") as yps, \
         tc.tile_pool(name="ysb", bufs=3) as ysb:
        for m in range(NB128):
            msl = slice(128 * m, 128 * (m + 1))
            y_ps = yps.tile([128, 1024], F32, tag="y")
            for nh in range(2):
                nsl = slice(512 * nh, 512 * (nh + 1))
                nc.tensor.matmul(y_ps[:, nsl], outT[:, msl], wo_sb[:, nsl],
                                 start=True, stop=True)
            y_sb = ysb.tile([128, 1024], F32, tag="ysb")
            nc.vector.tensor_copy(y_sb[:], y_ps[:])
            nc.sync.dma_start(y.ap()[msl, :], y_sb[:])

    for p in reversed(ctx_pools):
        p.__exit__(None, None, None)


_CACHED = None


def _build():
    global _CACHED
    if _CACHED is not None:
        return _CACHED
    nc = bacc.Bacc("TRN2", target_bir_lowering=False, debug=False)
    xT = nc.dram_tensor("xT", [D, S], F32R, kind="ExternalInput")
    wqkT = nc.dram_tensor("wqkT", [D, 256], F32R, kind="ExternalInput")
    wvT = nc.dram_tensor("wvT", [D, 128], F32R, kind="ExternalInput")
    woT = nc.dram_tensor("woT", [128, D], F32R, kind="ExternalInput")
    cosT = nc.dram_tensor("cosT", [128, S], F32, kind="ExternalInput")
    sinT = nc.dram_tensor("sinT", [128, S], F32, kind="ExternalInput")
    tri = nc.dram_tensor("tri", [128, 512], F32, kind="ExternalInput")
    y = nc.dram_tensor("y", [S, D], F32, kind="ExternalOutput")
    with tile.TileContext(nc) as tc:
        _emit(tc, xT, wqkT, wvT, woT, cosT, sinT, tri, y)
    nc.compile()
    _CACHED = nc
    return nc


def _host_prep(x, token_positions, Wq, Wk, Wv, Wo):
    x = np.asarray(x, dtype=np.float32).reshape(S, D)
    xT = np.ascontiguousarray(x.T)

    pos = np.asarray(token_positions).reshape(S).astype(np.float32)
    inv = (np.float32(10000.0) **
           (-np.arange(0, DK // 2, dtype=np.float32) * np.float32(2.0 / DK)))
    ang = pos[None, :] * inv[:, None]          # [32, S]
    cosF = np.cos(ang).astype(np.float32)
    sinF = np.sin(ang).astype(np.float32)
    cosT = np.ascontiguousarray(np.tile(cosF, (4, 1)))  # [128, S]
    sinT = np.ascontiguousarray(np.tile(sinF, (4, 1)))

    ii = np.arange(128)[:, None]
    uu = np.arange(512)[None, :]
    tri = (uu >= ii + 384).astype(np.float32)   # strip mask B01 [128, 512]

    Wq = np.asarray(Wq, dtype=np.float32)
    Wk = np.asarray(Wk, dtype=np.float32)
    Wv = np.asarray(Wv, dtype=np.float32)
    Wo = np.asarray(Wo, dtype=np.float32)

    in_maps = []
    for c in range(N_CORES):
        idx = []
        for hl in range(2):   # per head: 32 even channels then 32 odd channels
            idx += [64 * (2 * c + hl) + 2 * j for j in range(32)]
            idx += [64 * (2 * c + hl) + 2 * j + 1 for j in range(32)]
        wq_c = Wq[idx, :]                       # [128, 1024]
        wk_c = Wk[idx, :]
        wqkT = np.ascontiguousarray(
            np.concatenate([wq_c.T, wk_c.T], axis=1))      # [1024, 256]
        wvT = np.ascontiguousarray(Wv[128 * c:128 * (c + 1), :].T)  # [1024, 128]
        woT = np.ascontiguousarray(Wo[:, 128 * c:128 * (c + 1)].T)  # [128, 1024]
        in_maps.append({
            "xT": xT, "wqkT": wqkT, "wvT": wvT, "woT": woT,
            "cosT": cosT, "sinT": sinT, "tri": tri,
        })
    return in_maps


def run(x, token_positions, Wq, Wk, Wv, Wo, trace=False):
    nc = _build()
    in_maps = _host_prep(x, token_positions, Wq, Wk, Wv, Wo)
    res = run_bass_kernel_spmd(nc, in_maps, core_ids=list(range(N_CORES)),
                               trace=trace)
    y = np.zeros((S, D), dtype=np.float32)
    for c in range(N_CORES):
        y += res.results[c]["y"]
    return y.reshape(1, S, D), res


def kernel(x, token_positions, Wq, Wk, Wv, Wo):
    y, _ = run(x, token_positions, Wq, Wk, Wv, Wo)
    return y


# revision 18
# speedup vs baseline: 1.0663x; 1.0663x over previous
"""Causal multi-head self-attention (B=1, S=4096, D=1024, H=16) on 8 NeuronCores.

Sharding: tensor-parallel over heads — each core owns 2 heads (Wq/Wk/Wv column
slices, Wo row slice), computes a partial output projection, and the host sums
the 8 partials.

Device-side design (per core):
  - Host pre-transposes x -> xT [D, S] and pre-permutes/stacks weights so no
    on-device layout shuffles are needed for q/k.
  - qT/kT computed in [channel, seq] layout (rows = [h0_x1|h1_x1|h0_x2|h1_x2],
    channels de-interleaved even/odd so RoPE is two contiguous 64-row halves).
  - v computed in [ch, seq] then PE-transposed to [seq, ch] blocks (PV matmul
    needs V as the stationary operand in natural layout).
  - Attention in scores^T layout: scores^T[sk,sq] = K_blk @ Q_blk^T, softmax
    without max-subtraction (scores are O(±8) for this distribution so exp is
    safe in fp32), row-sums obtained by appending a ones-column to V, causal
    masking via one static 128x128 triangle tile + gpsimd memsets on fully
    masked column ranges.
  - All matmuls run as float32r (full PE rate, fp32 storage).
"""

import os
import sys

import numpy as np

for _p in ("/opt/trn_rl_repo", "/root/.axon_site/_ro/trn_rl_repo"):
    if os.path.isdir(_p) and _p not in sys.path:
        sys.path.insert(0, _p)

import concourse.bass as bass
import concourse.mybir as mybir
import concourse.tile as tile
from concourse import bacc
from concourse.bass_utils import run_bass_kernel_spmd
from concourse.masks import make_identity


def _install_ntff_shim():
    """The agent image's antenv lacks axon_hooks; provide it so
    run_bass_kernel_spmd(trace=True) can capture NTFF profiles."""
    try:
        from antenv import axon_hooks  # noqa: F401
        return
    except ImportError:
        pass
    try:
        import types
        import antenv
        from trn_agent_boot.trn_boot import _ntff_profile_via_ctypes
        so = "/opt/axon/libaxon_pjrt.so"
        if not os.path.exists(so):
            return
        hook = _ntff_profile_via_ctypes(so)
        m = types.ModuleType("antenv.axon_hooks")
        m._hook = hook
        m.set_axon_ntff_profile_hook = lambda h: setattr(m, "_hook", h)
        m.get_axon_ntff_profile_hook = lambda: m._hook
        sys.modules["antenv.axon_hooks"] = m
        antenv.axon_hooks = m
    except Exception:
        pass


_install_ntff_shim()

F32 = mybir.dt.float32
F32R = mybir.dt.float32r

S = 4096
D = 1024
H = 16
DK = 64
N_CORES = 8
SQ = 512          # q-block width (PSUM bank limit for fp32 N)
SK = 128          # k-block width (partition dim of scores^T)
GK = 3            # k-blocks fused per exp group (3 PSUM banks)
NB512 = S // SQ   # 8
NB128 = S // SK   # 32


def _emit(tc, xT, wqkT, wvT, woT, cosT, sinT, tri, y):
    nc = tc.nc
    ctx_pools = []
    PHASES = int(os.environ.get("KERN_PHASES", "5"))

    # ---------------- persistent SBUF ----------------
    const = tc.tile_pool(name="const", bufs=1)
    big = tc.tile_pool(name="big", bufs=1)
    cp = const.__enter__()
    bp = big.__enter__()
    ctx_pools += [const, big]

    wqk_sb = cp.tile([128, 8, 256], F32R, tag="wqk")      # [part, kchunk, 256]
    wv_sb = cp.tile([128, 8, 128], F32R, tag="wv")
    wo_sb = cp.tile([128, 1024], F32R, tag="wo")
    cos_sb = cp.tile([128, S], F32, tag="cos")
    sin_sb = cp.tile([128, S], F32, tag="sin")
    tri_sb = cp.tile([128, 512], F32, tag="tri")
    ident = cp.tile([128, 128], F32, tag="ident")
    ones_sb = cp.tile([65, 64], F32R, tag="ones")

    nc.sync.dma_start(wqk_sb[:], wqkT.ap().rearrange("(c p) n -> p c n", p=128))
    nc.sync.dma_start(wv_sb[:], wvT.ap().rearrange("(c p) n -> p c n", p=128))
    nc.sync.dma_start(wo_sb[:], woT.ap())
    nc.sync.dma_start(cos_sb[:], cosT.ap())
    nc.sync.dma_start(sin_sb[:], sinT.ap())
    nc.sync.dma_start(tri_sb[:], tri.ap())
    make_identity(nc, ident[:])
    ones_f32 = cp.tile([65, 64], F32, tag="ones_f32")
    nc.vector.memset(ones_f32[:], 1.0)
    nc.vector.tensor_copy(ones_sb[64:65, :], ones_f32[64:65, :])

    qT = bp.tile([128, S], F32R, tag="qT")
    kT = bp.tile([128, S], F32R, tag="kT")
    vt_sb = bp.tile([128, S], F32, tag="vt_out")         # vT, later reused for outT
    v_h = [bp.tile([128, NB128 * 65], F32R, tag=f"v{h}", name=f"v{h}")
           for h in range(2)]

    # ---------------- phase B: projections ----------------
    with tc.tile_pool(name="xp", bufs=12) as xp, \
         tc.tile_pool(name="prps", bufs=6, space="PSUM") as prps:
        for sb in range(NB512):
            xts = []
            for kc in range(8):
                xt = xp.tile([128, SQ], F32R, tag="x")
                nc.sync.dma_start(
                    xt[:], xT.ap()[128 * kc:128 * (kc + 1), SQ * sb:SQ * (sb + 1)])
                xts.append(xt)
            q_ps = prps.tile([128, SQ], F32, tag="pr")
            k_ps = prps.tile([128, SQ], F32, tag="pr")
            v_ps = prps.tile([128, SQ], F32, tag="pr")
            for kc in range(8):
                st, sp = (kc == 0), (kc == 7)
                nc.tensor.matmul(q_ps[:], wqk_sb[:, kc, 0:128], xts[kc][:],
                                 start=st, stop=sp)
                nc.tensor.matmul(k_ps[:], wqk_sb[:, kc, 128:256], xts[kc][:],
                                 start=st, stop=sp)
                nc.tensor.matmul(v_ps[:], wv_sb[:, kc, :], xts[kc][:],
                                 start=st, stop=sp)
            sl = slice(SQ * sb, SQ * (sb + 1))
            nc.vector.tensor_copy(qT[:, sl], q_ps[:])
            nc.vector.tensor_copy(kT[:, sl], k_ps[:])
            nc.vector.tensor_copy(vt_sb[:, sl], v_ps[:])

    if PHASES < 2:
        for p in reversed(ctx_pools):
            p.__exit__(None, None, None)
        return
    # ---------------- RoPE (in place on qT, kT) ----------------
    # Row layout: [h0_x1(32) | h0_x2(32) | h1_x1(32) | h1_x2(32)] so each
    # head's d_k=64 rows are contiguous for the attention matmuls.
    # sw = 32-row swap within each head block; the RoPE +- sign is folded
    # into the host-built sin table (x1 rows carry -sin), so the whole
    # update is three full-width partition-local ops:
    #   ta = t * cos2;  tb = sw * sin2_signed;  t = ta + tb
    mul = mybir.AluOpType.mult
    with tc.tile_pool(name="rope", bufs=2) as rp:
        HW = 2048  # free-dim chunk
        sws = {}
        for t in (qT, kT):
            sw = rp.tile([128, S], F32R, tag=f"swp{len(sws)}",
                         name=f"swp{len(sws)}", bufs=1)
            sws[id(t)] = sw
            for blk in range(4):
                dst = slice(32 * blk, 32 * blk + 32)
                srcs = slice(32 * (blk ^ 1), 32 * (blk ^ 1) + 32)
                nc.sync.dma_start(sw[dst, :], t[srcs, :])
        for c0 in range(0, S, HW):
            sl = slice(c0, c0 + HW)
            for t in (qT, kT):
                sw = sws[id(t)]
                ta = rp.tile([128, HW], F32, tag="ta")
                tb = rp.tile([128, HW], F32, tag="tb")
                nc.vector.tensor_tensor(ta[:], t[:, sl], cos_sb[:, sl], op=mul)
                nc.vector.tensor_tensor(tb[:], sw[:, sl], sin_sb[:, sl], op=mul)
                nc.vector.tensor_tensor(t[:, sl], ta[:], tb[:],
                                        op=mybir.AluOpType.add)

    if PHASES < 3:
        for p in reversed(ctx_pools):
            p.__exit__(None, None, None)
        return
    # ---------------- phase C: v^T -> v blocks (PE transpose) ----------------
    # ones column for row-sums: v_h[:, 65j+64] = 1.0
    onec = cp.tile([128, 1], F32, tag="onec")
    nc.vector.memset(onec[:], 1.0)
    for h in range(2):
        ones_col = v_h[h][:].rearrange("p (b c) -> p b c", c=65)[:, :, 64]
        nc.vector.tensor_copy(ones_col, onec[:].broadcast_to([128, 32]))
    with tc.tile_pool(name="vtp", bufs=4, space="PSUM") as vtp:
        for j in range(NB128):
            tp = vtp.tile([128, 128], F32, tag="vt")
            nc.tensor.transpose(tp[:], vt_sb[:, 128 * j:128 * (j + 1)], ident[:])
            for h in range(2):
                nc.vector.tensor_copy(v_h[h][:, 65 * j:65 * j + 64],
                                      tp[:, 64 * h:64 * h + 64])

    if PHASES < 4:
        for p in reversed(ctx_pools):
            p.__exit__(None, None, None)
        return
    # ---------------- phase D: attention ----------------
    DMODE = int(os.environ.get("KERN_DMODE", "4"))
    outT = bp.tile([128, S], F32R, tag="vt_out")  # reuses vT slot
    exp = mybir.ActivationFunctionType.Exp
    with tc.tile_pool(name="scps", bufs=2, space="PSUM") as scps, \
         tc.tile_pool(name="smps", bufs=2, space="PSUM") as smps, \
         tc.tile_pool(name="ptp", bufs=3) as ptp, \
         tc.tile_pool(name="recp", bufs=2) as recp:
        for b in range(NB512):
            nk = 4 * b + 4
            qsl = slice(SQ * b, SQ * (b + 1))
            for h in range(2):
                out_ps = smps.tile([65, SQ], F32, tag="sm")
                rh = slice(64 * h, 64 * h + 64)          # head h d_k rows
                for g0 in range(0, nk, GK):
                    gw = min(GK, nk - g0)
                    sc = scps.tile([128, GK * SQ], F32, tag="sc")
                    for j2 in range(gw):
                        k = g0 + j2
                        ksl = slice(SK * k, SK * (k + 1))
                        osl = slice(SQ * j2, SQ * (j2 + 1))
                        nc.tensor.matmul(sc[:, osl], kT[rh, ksl], qT[rh, qsl],
                                         start=True, stop=True)
                    pt = ptp.tile([128, GK * SQ], F32R, tag="pt")
                    nc.scalar.activation(pt[:, 0:SQ * gw], sc[:, 0:SQ * gw], exp,
                                         scale=0.125)
                    if DMODE >= 2:
                        for j2 in range(gw):
                            k = g0 + j2
                            if k >= 4 * b:           # diagonal-straddling block
                                j = k - 4 * b
                                c0 = SQ * j2
                                w = 128 * j + 128
                                dsl = slice(c0, c0 + w)
                                nc.vector.tensor_tensor(pt[:, dsl], pt[:, dsl],
                                                        tri_sb[:, 512 - w:512],
                                                        op=mul)
                    if DMODE >= 3:
                        for j2 in range(gw):
                            k = g0 + j2
                            nc.tensor.matmul(out_ps[:],
                                             v_h[h][:, 65 * k:65 * k + 65],
                                             pt[:, SQ * j2:SQ * (j2 + 1)],
                                             start=(k == 0), stop=(k == nk - 1))
                if DMODE < 4:
                    continue
                rec = recp.tile([65, SQ], F32R, tag="rec")
                with nc.allow_low_precision(reason="fp32r reciprocal for bcast"):
                    nc.vector.reciprocal(rec[64:65, :], out_ps[64:65, :])
                bc = smps.tile([64, SQ], F32, tag="sm")
                nc.tensor.matmul(bc[:], ones_sb[64:65, :], rec[64:65, :],
                                 start=True, stop=True)
                if h == 0:
                    dst = outT[0:64, qsl]
                else:
                    tmp64 = recp.tile([64, SQ], F32R, tag="tmp64")
                    dst = tmp64[:]
                nc.vector.tensor_copy(dst, out_ps[0:64, :])
                nc.vector.tensor_tensor(dst, dst, bc[:], op=mul)
                if h == 1:
                    nc.sync.dma_start(outT[64:128, qsl], tmp64[:])

    if PHASES < 5:
        for p in reversed(ctx_pools):
            p.__exit__(None, None, None)
        return
    # ---------------- phase E: output projection ----------------
    with tc.tile_pool(name="yps", bufs=2, space="PSUM") as yps, \
         tc.tile_pool(name="ysb", bufs=3) as ysb:
        for m in range(NB128):
            msl = slice(128 * m, 128 * (m + 1))
            y_ps = yps.tile([128, 1024], F32, tag="y")
            for nh in range(2):
                nsl = slice(512 * nh, 512 * (nh + 1))
                nc.tensor.matmul(y_ps[:, nsl], outT[:, msl], wo_sb[:, nsl],
                                 start=True, stop=True)
            y_sb = ysb.tile([128, 1024], F32, tag="ysb")
            if m % 2 == 0:
                nc.vector.tensor_copy(y_sb[:], y_ps[:])
            else:
                nc.scalar.copy(y_sb[:], y_ps[:])
            nc.sync.dma_start(y.ap()[msl, :], y_sb[:])

    for p in reversed(ctx_pools):
        p.__exit__(None, None, None)


_CACHED = None


def _build():
    global _CACHED
    if _CACHED is not None:
        return _CACHED
    nc = bacc.Bacc("TRN2", target_bir_lowering=False, debug=False)
    xT = nc.dram_tensor("xT", [D, S], F32R, kind="ExternalInput")
    wqkT = nc.dram_tensor("wqkT", [D, 256], F32R, kind="ExternalInput")
    wvT = nc.dram_tensor("wvT", [D, 128], F32R, kind="ExternalInput")
    woT = nc.dram_tensor("woT", [128, D], F32R, kind="ExternalInput")
    cosT = nc.dram_tensor("cosT", [128, S], F32, kind="ExternalInput")
    sinT = nc.dram_tensor("sinT", [128, S], F32, kind="ExternalInput")
    tri = nc.dram_tensor("tri", [128, 512], F32, kind="ExternalInput")
    y = nc.dram_tensor("y", [S, D], F32, kind="ExternalOutput")
    with tile.TileContext(nc) as tc:
        _emit(tc, xT, wqkT, wvT, woT, cosT, sinT, tri, y)
    nc.compile()
    _CACHED = nc
    return nc


def _host_prep(x, token_positions, Wq, Wk, Wv, Wo):
    x = np.asarray(x, dtype=np.float32).reshape(S, D)
    xT = np.ascontiguousarray(x.T)

    pos = np.asarray(token_positions).reshape(S).astype(np.float32)
    inv = (np.float32(10000.0) **
           (-np.arange(0, DK // 2, dtype=np.float32) * np.float32(2.0 / DK)))
    ang = pos[None, :] * inv[:, None]          # [32, S]
    cosF = np.cos(ang).astype(np.float32)
    sinF = np.sin(ang).astype(np.float32)
    cosT = np.ascontiguousarray(np.tile(cosF, (4, 1)))          # [128, S]
    sinT = np.ascontiguousarray(np.tile(
        np.concatenate([-sinF, sinF], axis=0), (2, 1)))          # signed

    ii = np.arange(128)[:, None]
    uu = np.arange(512)[None, :]
    tri = (uu >= ii + 384).astype(np.float32)   # strip mask B01 [128, 512]

    Wq = np.asarray(Wq, dtype=np.float32)
    Wk = np.asarray(Wk, dtype=np.float32)
    Wv = np.asarray(Wv, dtype=np.float32)
    Wo = np.asarray(Wo, dtype=np.float32)

    in_maps = []
    for c in range(N_CORES):
        idx = []
        for hl in range(2):   # per head: 32 even channels then 32 odd channels
            idx += [64 * (2 * c + hl) + 2 * j for j in range(32)]
            idx += [64 * (2 * c + hl) + 2 * j + 1 for j in range(32)]
        wq_c = Wq[idx, :]                       # [128, 1024]
        wk_c = Wk[idx, :]
        wqkT = np.ascontiguousarray(
            np.concatenate([wq_c.T, wk_c.T], axis=1))      # [1024, 256]
        wvT = np.ascontiguousarray(Wv[128 * c:128 * (c + 1), :].T)  # [1024, 128]
        woT = np.ascontiguousarray(Wo[:, 128 * c:128 * (c + 1)].T)  # [128, 1024]
        in_maps.append({
            "xT": xT, "wqkT": wqkT, "wvT": wvT, "woT": woT,
            "cosT": cosT, "sinT": sinT, "tri": tri,
        })
    return in_maps


def run(x, token_positions, Wq, Wk, Wv, Wo, trace=False):
    nc = _build()
    in_maps = _host_prep(x, token_positions, Wq, Wk, Wv, Wo)
    res = run_bass_kernel_spmd(nc, in_maps, core_ids=list(range(N_CORES)),
                               trace=trace)
    y = np.zeros((S, D), dtype=np.float32)
    for c in range(N_CORES):
        y += res.results[c]["y"]
    return y.reshape(1, S, D), res


def kernel(x, token_positions, Wq, Wk, Wv, Wo):
    y, _ = run(x, token_positions, Wq, Wk, Wv, Wo)
    return y


# revision 22
# speedup vs baseline: 1.1101x; 1.0411x over previous
"""Causal multi-head self-attention (B=1, S=4096, D=1024, H=16) on 8 NeuronCores.

Sharding: tensor-parallel over heads — each core owns 2 heads (Wq/Wk/Wv column
slices, Wo row slice), computes a partial output projection, and the host sums
the 8 partials.

Device-side design (per core):
  - Host pre-transposes x -> xT [D, S] and pre-permutes/stacks weights so no
    on-device layout shuffles are needed for q/k.
  - qT/kT computed in [channel, seq] layout (rows = [h0_x1|h1_x1|h0_x2|h1_x2],
    channels de-interleaved even/odd so RoPE is two contiguous 64-row halves).
  - v computed in [ch, seq] then PE-transposed to [seq, ch] blocks (PV matmul
    needs V as the stationary operand in natural layout).
  - Attention in scores^T layout: scores^T[sk,sq] = K_blk @ Q_blk^T, softmax
    without max-subtraction (scores are O(±8) for this distribution so exp is
    safe in fp32), row-sums obtained by appending a ones-column to V, causal
    masking via one static 128x128 triangle tile + gpsimd memsets on fully
    masked column ranges.
  - All matmuls run as float32r (full PE rate, fp32 storage).
"""

import os
import sys

import numpy as np

for _p in ("/opt/trn_rl_repo", "/root/.axon_site/_ro/trn_rl_repo"):
    if os.path.isdir(_p) and _p not in sys.path:
        sys.path.insert(0, _p)

import concourse.bass as bass
import concourse.mybir as mybir
import concourse.tile as tile
from concourse import bacc
from concourse.bass_utils import run_bass_kernel_spmd
from concourse.masks import make_identity


def _install_ntff_shim():
    """The agent image's antenv lacks axon_hooks; provide it so
    run_bass_kernel_spmd(trace=True) can capture NTFF profiles."""
    try:
        from antenv import axon_hooks  # noqa: F401
        return
    except ImportError:
        pass
    try:
        import types
        import antenv
        from trn_agent_boot.trn_boot import _ntff_profile_via_ctypes
        so = "/opt/axon/libaxon_pjrt.so"
        if not os.path.exists(so):
            return
        hook = _ntff_profile_via_ctypes(so)
        m = types.ModuleType("antenv.axon_hooks")
        m._hook = hook
        m.set_axon_ntff_profile_hook = lambda h: setattr(m, "_hook", h)
        m.get_axon_ntff_profile_hook = lambda: m._hook
        sys.modules["antenv.axon_hooks"] = m
        antenv.axon_hooks = m
    except Exception:
        pass


_install_ntff_shim()

F32 = mybir.dt.float32
F32R = mybir.dt.float32r

S = 4096
D = 1024
H = 16
DK = 64
N_CORES = 8
SQ = 512          # q-block width (PSUM bank limit for fp32 N)
SK = 128          # k-block width (partition dim of scores^T)
GK = 2            # k-blocks fused per exp group (2 PSUM banks)
NB512 = S // SQ   # 8
NB128 = S // SK   # 32


def _emit(tc, xT, wqkT, wvT, woT, cosT, sinT, tri, y):
    nc = tc.nc
    ctx_pools = []
    PHASES = int(os.environ.get("KERN_PHASES", "5"))

    # ---------------- persistent SBUF ----------------
    const = tc.tile_pool(name="const", bufs=1)
    big = tc.tile_pool(name="big", bufs=1)
    cp = const.__enter__()
    bp = big.__enter__()
    ctx_pools += [const, big]

    wqk_sb = cp.tile([128, 8, 256], F32R, tag="wqk")      # [part, kchunk, 256]
    wv_sb = cp.tile([128, 8, 128], F32R, tag="wv")
    wo_sb = cp.tile([128, 1024], F32R, tag="wo")
    cos_sb = cp.tile([128, S], F32, tag="cos")
    sin_sb = cp.tile([128, S], F32, tag="sin")
    tri_sb = cp.tile([128, 512], F32, tag="tri")
    ident = cp.tile([128, 128], F32, tag="ident")
    ones_sb = cp.tile([65, 64], F32R, tag="ones")

    nc.sync.dma_start(wqk_sb[:], wqkT.ap().rearrange("(c p) n -> p c n", p=128))
    nc.sync.dma_start(wv_sb[:], wvT.ap().rearrange("(c p) n -> p c n", p=128))
    nc.sync.dma_start(wo_sb[:], woT.ap())
    nc.sync.dma_start(cos_sb[:], cosT.ap())
    nc.sync.dma_start(sin_sb[:], sinT.ap())
    nc.sync.dma_start(tri_sb[:], tri.ap())
    make_identity(nc, ident[:])
    ones_f32 = cp.tile([65, 64], F32, tag="ones_f32")
    nc.vector.memset(ones_f32[:], 1.0)
    nc.vector.tensor_copy(ones_sb[64:65, :], ones_f32[64:65, :])

    qT = bp.tile([128, S], F32R, tag="qT")
    kT = bp.tile([128, S], F32R, tag="kT")
    vt_sb = bp.tile([128, S], F32, tag="vt_out")         # vT, later reused for outT
    v_h = [bp.tile([128, NB128 * 65], F32R, tag=f"v{h}", name=f"v{h}")
           for h in range(2)]

    # ---------------- phase B: projections ----------------
    with tc.tile_pool(name="xp", bufs=12) as xp, \
         tc.tile_pool(name="prps", bufs=6, space="PSUM") as prps:
        for sb in range(NB512):
            xts = []
            for kc in range(8):
                xt = xp.tile([128, SQ], F32R, tag="x")
                nc.sync.dma_start(
                    xt[:], xT.ap()[128 * kc:128 * (kc + 1), SQ * sb:SQ * (sb + 1)])
                xts.append(xt)
            q_ps = prps.tile([128, SQ], F32, tag="pr")
            k_ps = prps.tile([128, SQ], F32, tag="pr")
            v_ps = prps.tile([128, SQ], F32, tag="pr")
            for kc in range(8):
                st, sp = (kc == 0), (kc == 7)
                nc.tensor.matmul(q_ps[:], wqk_sb[:, kc, 0:128], xts[kc][:],
                                 start=st, stop=sp)
                nc.tensor.matmul(k_ps[:], wqk_sb[:, kc, 128:256], xts[kc][:],
                                 start=st, stop=sp)
                nc.tensor.matmul(v_ps[:], wv_sb[:, kc, :], xts[kc][:],
                                 start=st, stop=sp)
            sl = slice(SQ * sb, SQ * (sb + 1))
            nc.vector.tensor_copy(qT[:, sl], q_ps[:])
            nc.vector.tensor_copy(kT[:, sl], k_ps[:])
            nc.vector.tensor_copy(vt_sb[:, sl], v_ps[:])

    if PHASES < 2:
        for p in reversed(ctx_pools):
            p.__exit__(None, None, None)
        return
    # ---------------- phase C: v^T -> v blocks (PE transpose) ----------------
    # ones column for row-sums: v_h[:, 65j+64] = 1.0
    onec = cp.tile([128, 1], F32, tag="onec")
    nc.vector.memset(onec[:], 1.0)
    for h in range(2):
        ones_col = v_h[h][:].rearrange("p (b c) -> p b c", c=65)[:, :, 64]
        nc.vector.tensor_copy(ones_col, onec[:].broadcast_to([128, 32]))
    with tc.tile_pool(name="vtp", bufs=4, space="PSUM") as vtp:
        for j in range(NB128):
            tp = vtp.tile([128, 128], F32, tag="vt")
            nc.tensor.transpose(tp[:], vt_sb[:, 128 * j:128 * (j + 1)], ident[:])
            for h in range(2):
                nc.vector.tensor_copy(v_h[h][:, 65 * j:65 * j + 64],
                                      tp[:, 64 * h:64 * h + 64])

    if PHASES < 3:
        for p in reversed(ctx_pools):
            p.__exit__(None, None, None)
        return
    # ---------------- RoPE (in place on qT, kT) ----------------
    # Row layout: [h0_x1(32) | h0_x2(32) | h1_x1(32) | h1_x2(32)] so each
    # head's d_k=64 rows are contiguous for the attention matmuls.
    # sw = 32-row swap within each head block; the RoPE +- sign is folded
    # into the host-built sin table (x1 rows carry -sin), so the whole
    # update is three full-width partition-local ops:
    #   ta = t * cos2;  tb = sw * sin2_signed;  t = ta + tb
    mul = mybir.AluOpType.mult
    with tc.tile_pool(name="rope", bufs=2) as rp:
        HW = 2048  # free-dim chunk
        sws = {}
        for t in (qT, kT):
            sw = rp.tile([128, S], F32R, tag=f"swp{len(sws)}",
                         name=f"swp{len(sws)}", bufs=1)
            sws[id(t)] = sw
        for c0 in range(0, S, HW):
            sl = slice(c0, c0 + HW)
            for t in (qT, kT):
                sw = sws[id(t)]
                for blk in range(4):
                    dst = slice(32 * blk, 32 * blk + 32)
                    srcs = slice(32 * (blk ^ 1), 32 * (blk ^ 1) + 32)
                    nc.sync.dma_start(sw[dst, sl], t[srcs, sl])
                ta = rp.tile([128, HW], F32, tag="ta")
                tb = rp.tile([128, HW], F32, tag="tb")
                nc.vector.tensor_tensor(ta[:], t[:, sl], cos_sb[:, sl], op=mul)
                nc.vector.tensor_tensor(tb[:], sw[:, sl], sin_sb[:, sl], op=mul)
                nc.vector.tensor_tensor(t[:, sl], ta[:], tb[:],
                                        op=mybir.AluOpType.add)

    if PHASES < 3:
        for p in reversed(ctx_pools):
            p.__exit__(None, None, None)
        return
    if PHASES < 4:
        for p in reversed(ctx_pools):
            p.__exit__(None, None, None)
        return
    # ---------------- phase D: attention + interleaved output projection ----
    DMODE = int(os.environ.get("KERN_DMODE", "4"))
    outT = bp.tile([128, S], F32R, tag="vt_out")  # reuses vT slot
    exp = mybir.ActivationFunctionType.Exp
    with tc.tile_pool(name="scps", bufs=2, space="PSUM") as scps, \
         tc.tile_pool(name="smps", bufs=1, space="PSUM") as smps, \
         tc.tile_pool(name="yps", bufs=3, space="PSUM") as yps, \
         tc.tile_pool(name="ptp", bufs=3) as ptp, \
         tc.tile_pool(name="ysb", bufs=4) as ysb, \
         tc.tile_pool(name="recp", bufs=2) as recp:
        pending = [None]

        def flush_norm():
            if pending[0] is None:
                return
            h_, b_, out_ps_ = pending[0]
            pending[0] = None
            qsl_ = slice(SQ * b_, SQ * (b_ + 1))
            rec = recp.tile([65, SQ], F32R, tag="rec", name="rec")
            with nc.allow_low_precision(reason="fp32r reciprocal for bcast"):
                nc.vector.reciprocal(rec[64:65, :], out_ps_[64:65, :])
            bc = yps.tile([64, SQ], F32, tag="y", name="bc")
            nc.tensor.matmul(bc[:], ones_sb[64:65, :], rec[64:65, :],
                             start=True, stop=True)
            if h_ == 0:
                dst = outT[0:64, qsl_]
                nc.vector.tensor_copy(dst, out_ps_[0:64, :])
                nc.vector.tensor_tensor(dst, dst, bc[:], op=mul)
            else:
                tmp64 = recp.tile([64, SQ], F32R, tag="tmp64", name="tmp64")
                nc.vector.tensor_copy(tmp64[:], out_ps_[0:64, :])
                nc.vector.tensor_tensor(tmp64[:], tmp64[:], bc[:], op=mul)
                nc.sync.dma_start(outT[64:128, qsl_], tmp64[:])
                # outT block b_ now complete -> emit its output projection
                for m in range(4 * b_, 4 * b_ + 4):
                    msl = slice(128 * m, 128 * (m + 1))
                    for nh in range(2):
                        nsl = slice(512 * nh, 512 * (nh + 1))
                        y_ps = yps.tile([128, SQ], F32, tag="y", name="y_ps")
                        nc.tensor.matmul(y_ps[:], outT[:, msl], wo_sb[:, nsl],
                                         start=True, stop=True)
                        y_sb = ysb.tile([128, SQ], F32, tag="ysb", name="y_sb")
                        if (m + nh) % 2 == 0:
                            nc.vector.tensor_copy(y_sb[:], y_ps[:])
                        else:
                            nc.scalar.copy(y_sb[:], y_ps[:])
                        nc.sync.dma_start(y.ap()[msl, nsl], y_sb[:])

        for b in range(NB512):
            nk = 4 * b + 4
            qsl = slice(SQ * b, SQ * (b + 1))
            for h in range(2):
                out_ps = smps.tile([65, SQ], F32, tag="sm")
                rh = slice(64 * h, 64 * h + 64)          # head h d_k rows
                for g0 in range(0, nk, GK):
                    gw = min(GK, nk - g0)
                    sc = scps.tile([128, GK * SQ], F32, tag="sc")
                    for j2 in range(gw):
                        k = g0 + j2
                        ksl = slice(SK * k, SK * (k + 1))
                        osl = slice(SQ * j2, SQ * (j2 + 1))
                        nc.tensor.matmul(sc[:, osl], kT[rh, ksl], qT[rh, qsl],
                                         start=True, stop=True)
                    pt = ptp.tile([128, GK * SQ], F32R, tag="pt")
                    nc.scalar.activation(pt[:, 0:SQ * gw], sc[:, 0:SQ * gw], exp,
                                         scale=0.125)
                    for j2 in range(gw):
                        k = g0 + j2
                        if k >= 4 * b:               # diagonal-straddling block
                            j = k - 4 * b
                            c0 = SQ * j2
                            w = 128 * j + 128
                            dsl = slice(c0, c0 + w)
                            nc.vector.tensor_tensor(pt[:, dsl], pt[:, dsl],
                                                    tri_sb[:, 512 - w:512],
                                                    op=mul)
                    if g0 == 0:
                        flush_norm()
                    for j2 in range(gw):
                        k = g0 + j2
                        nc.tensor.matmul(out_ps[:],
                                         v_h[h][:, 65 * k:65 * k + 65],
                                         pt[:, SQ * j2:SQ * (j2 + 1)],
                                         start=(k == 0), stop=(k == nk - 1))
                pending[0] = (h, b, out_ps)
        flush_norm()

    for p in reversed(ctx_pools):
        p.__exit__(None, None, None)


_CACHED = None


def _build():
    global _CACHED
    if _CACHED is not None:
        return _CACHED
    nc = bacc.Bacc("TRN2", target_bir_lowering=False, debug=False)
    xT = nc.dram_tensor("xT", [D, S], F32R, kind="ExternalInput")
    wqkT = nc.dram_tensor("wqkT", [D, 256], F32R, kind="ExternalInput")
    wvT = nc.dram_tensor("wvT", [D, 128], F32R, kind="ExternalInput")
    woT = nc.dram_tensor("woT", [128, D], F32R, kind="ExternalInput")
    cosT = nc.dram_tensor("cosT", [128, S], F32, kind="ExternalInput")
    sinT = nc.dram_tensor("sinT", [128, S], F32, kind="ExternalInput")
    tri = nc.dram_tensor("tri", [128, 512], F32, kind="ExternalInput")
    y = nc.dram_tensor("y", [S, D], F32, kind="ExternalOutput")
    with tile.TileContext(nc) as tc:
        _emit(tc, xT, wqkT, wvT, woT, cosT, sinT, tri, y)
    nc.compile()
    _CACHED = nc
    return nc


def _host_prep(x, token_positions, Wq, Wk, Wv, Wo):
    x = np.asarray(x, dtype=np.float32).reshape(S, D)
    xT = np.ascontiguousarray(x.T)

    pos = np.asarray(token_positions).reshape(S).astype(np.float32)
    inv = (np.float32(10000.0) **
           (-np.arange(0, DK // 2, dtype=np.float32) * np.float32(2.0 / DK)))
    ang = pos[None, :] * inv[:, None]          # [32, S]
    cosF = np.cos(ang).astype(np.float32)
    sinF = np.sin(ang).astype(np.float32)
    cosT = np.ascontiguousarray(np.tile(cosF, (4, 1)))          # [128, S]
    sinT = np.ascontiguousarray(np.tile(
        np.concatenate([-sinF, sinF], axis=0), (2, 1)))          # signed

    ii = np.arange(128)[:, None]
    uu = np.arange(512)[None, :]
    tri = (uu >= ii + 384).astype(np.float32)   # strip mask B01 [128, 512]

    Wq = np.asarray(Wq, dtype=np.float32)
    Wk = np.asarray(Wk, dtype=np.float32)
    Wv = np.asarray(Wv, dtype=np.float32)
    Wo = np.asarray(Wo, dtype=np.float32)

    in_maps = []
    for c in range(N_CORES):
        idx = []
        for hl in range(2):   # per head: 32 even channels then 32 odd channels
            idx += [64 * (2 * c + hl) + 2 * j for j in range(32)]
            idx += [64 * (2 * c + hl) + 2 * j + 1 for j in range(32)]
        wq_c = Wq[idx, :]                       # [128, 1024]
        wk_c = Wk[idx, :]
        wqkT = np.ascontiguousarray(
            np.concatenate([wq_c.T, wk_c.T], axis=1))      # [1024, 256]
        wvT = np.ascontiguousarray(Wv[128 * c:128 * (c + 1), :].T)  # [1024, 128]
        woT = np.ascontiguousarray(Wo[:, 128 * c:128 * (c + 1)].T)  # [128, 1024]
        in_maps.append({
            "xT": xT, "wqkT": wqkT, "wvT": wvT, "woT": woT,
            "cosT": cosT, "sinT": sinT, "tri": tri,
        })
    return in_maps


def run(x, token_positions, Wq, Wk, Wv, Wo, trace=False):
    nc = _build()
    in_maps = _host_prep(x, token_positions, Wq, Wk, Wv, Wo)
    res = run_bass_kernel_spmd(nc, in_maps, core_ids=list(range(N_CORES)),
                               trace=trace)
    y = np.zeros((S, D), dtype=np.float32)
    for c in range(N_CORES):
        y += res.results[c]["y"]
    return y.reshape(1, S, D), res


def kernel(x, token_positions, Wq, Wk, Wv, Wo):
    y, _ = run(x, token_positions, Wq, Wk, Wv, Wo)
    return y
